# revision 4
# baseline (speedup 1.0000x reference)
"""Trainium2 Bass kernel for per-head causal attention (nn_Attention_52896817217709).

Sharding: 8 cores = 4 head-groups (3 heads each) x 2 batches.
Per core, per head h (S=2048, D_MODEL=768, D_HEAD=64):
  qT = W_Q[h].T @ Xq[h].T        (host supplies X pre-transposed: [768, 2048])
  kT, vT analogous
  S^T[k, q] = kT_chunk.T @ qT    (scores transposed: softmax-k on partitions)
  P = exp(0.125 * S^T)           (masked diagonal tile; strictly-upper tiles skipped)
  z'[d', q] = sum_k v'[k, d'].T @ P[k, q]   with v' = [v | 1] -> row 64 = softmax sums
  attn[q, m] = (z'^T_chunk.T @ [W_O; b_O/H]) * (1/sums[q])
All matmuls run as float32r (full-speed PE; operands rounded to 12-bit mantissa).
Head phases are software-pipelined (A=load+proj, B=attention loop, C=output):
A0 B0 A1 C0 B1 A2 C1 B2 C2 — keeps the PE stream dense so HAM stays warm.
"""
import sys
import os
import numpy as np

for _p in ("/opt/trn_rl_repo", "/root/.axon_site/_ro/trn_rl_repo"):
    if os.path.isdir(_p) and _p not in sys.path:
        sys.path.insert(0, _p)

import concourse.bass as bass
import concourse.tile as tile
from concourse import bacc, mybir
from concourse.bass_utils import run_bass_kernel_spmd

F32 = mybir.dt.float32
F32R = mybir.dt.float32r
BF16 = mybir.dt.bfloat16
FP16 = mybir.dt.float16
AF = mybir.ActivationFunctionType

B, S, H, DM, DH = 2, 2048, 12, 768, 64
HPC = 3            # heads per core
NT = S // 128      # 16 s-tiles
MT = DM // 128     # 6 m-tiles
N_CORES = 8


def _chunks_for(i):
    """(c, qlo, w) chunks of the causal q-range [128*i, 2048) split at 512 bounds."""
    out = []
    for c in range(4):
        qlo = max(512 * c, 128 * i)
        qhi = 512 * (c + 1)
        if qhi > qlo:
            out.append((c, qlo, qhi - qlo))
    return out


def build_program():
    nc = bacc.Bacc("TRN2", target_bir_lowering=False, debug=False)

    xq = nc.dram_tensor("xq", [HPC, DM, S], F32R, kind="ExternalInput")
    xk = nc.dram_tensor("xk", [HPC, DM, S], F32R, kind="ExternalInput")
    xv = nc.dram_tensor("xv", [HPC, DM, S], F32R, kind="ExternalInput")
    wq = nc.dram_tensor("wq", [HPC, MT, 128, DH], F32R, kind="ExternalInput")
    wk = nc.dram_tensor("wk", [HPC, MT, 128, DH], F32R, kind="ExternalInput")
    wv = nc.dram_tensor("wv", [HPC, MT, 128, DH], F32R, kind="ExternalInput")
    wo = nc.dram_tensor("wo", [HPC, DH + 1, DM], F32R, kind="ExternalInput")
    bq = nc.dram_tensor("bq", [HPC, DH, 1], F32, kind="ExternalInput")
    bk = nc.dram_tensor("bk", [HPC, DH, 1], F32, kind="ExternalInput")
    bv = nc.dram_tensor("bv", [HPC, DH, 1], F32, kind="ExternalInput")
    ident = nc.dram_tensor("ident", [128, 128], F32, kind="ExternalInput")
    maskd = nc.dram_tensor("maskd", [128, 128], FP16, kind="ExternalInput")
    ones16 = nc.dram_tensor("ones16", [128, NT], FP16, kind="ExternalInput")
    out = nc.dram_tensor("out", [HPC, S, DM], F32, kind="ExternalOutput")

    with tile.TileContext(nc) as tc:
        with (
            tc.tile_pool(name="wpool", bufs=1) as wpool,
            tc.tile_pool(name="xt", bufs=2) as xt_pool,
            tc.tile_pool(name="qk", bufs=2) as qk_pool,
            tc.tile_pool(name="vtp", bufs=1) as vt_pool,
            tc.tile_pool(name="vp", bufs=2) as vp_pool,
            tc.tile_pool(name="pp", bufs=2) as p_pool,
            tc.tile_pool(name="zt", bufs=1) as zt_pool,
            tc.tile_pool(name="sr", bufs=1) as sr_pool,
            tc.tile_pool(name="rc", bufs=2) as rc_pool,
            tc.tile_pool(name="ob", bufs=2) as out_pool,
            tc.tile_pool(name="ps_s", bufs=4, space="PSUM") as ps_s,
            tc.tile_pool(name="ps_a", bufs=2, space="PSUM") as ps_a,
            tc.tile_pool(name="ps_z", bufs=2, space="PSUM") as ps_z,
        ):
            # ---- preamble: weights/constants ----
            w_sb, b_sb = {}, {}
            for t, (wd, bd) in {"q": (wq, bq), "k": (wk, bk), "v": (wv, bv)}.items():
                for h in range(HPC):
                    wt = wpool.tile([128, MT, DH], F32R, name=f"w{t}{h}")
                    nc.sync.dma_start(wt[:], wd[h].rearrange("a p d -> p a d"))
                    bt = wpool.tile([DH, 1], F32, name=f"b{t}{h}")
                    nc.sync.dma_start(bt[:], bd[h])
                    w_sb[t, h] = wt
                    b_sb[t, h] = bt
            wo_sb = []
            for h in range(HPC):
                wot = wpool.tile([DH + 1, DM], F32R, name=f"wo{h}")
                nc.sync.dma_start(wot[:], wo[h])
                wo_sb.append(wot)
            id_sb = wpool.tile([128, 128], F32, name="id_sb")
            nc.sync.dma_start(id_sb[:], ident[:])
            mask_sb = wpool.tile([128, 128], FP16, name="mask_sb")
            nc.sync.dma_start(mask_sb[:], maskd[:])

            st = [dict() for _ in range(HPC)]   # per-head live tiles

            def emit_A(h):
                """Loads + projections + v' build for head h."""
                qT = qk_pool.tile([DH, S], F32R, name=f"qT{h}", tag="qT")
                kT = qk_pool.tile([DH, S], F32R, name=f"kT{h}", tag="kT")
                vT = vt_pool.tile([DH, S], F32, name=f"vT{h}", tag="vT")
                for t, xd, dst in (("q", xq, qT), ("k", xk, kT), ("v", xv, vT)):
                    halves = []
                    for n in range(2):
                        xh = xt_pool.tile([128, MT, S // 2], F32R,
                                          name=f"x{t}{h}_{n}", tag="xt")
                        nc.sync.dma_start(
                            xh[:],
                            xd[h].rearrange("(a p) s -> p a s", p=128)
                                 [:, :, bass.ts(n, S // 2)])
                        halves.append(xh)
                    for c in range(4):
                        acc = ps_s.tile([DH, 512], F32, name=f"acc{t}{h}{c}", tag="s")
                        src = halves[c // 2]
                        off = (c % 2) * 512
                        for mt in range(MT):
                            nc.tensor.matmul(
                                acc[:], w_sb[t, h][:, mt, :],
                                src[:, mt, off:off + 512],
                                start=(mt == 0), stop=(mt == MT - 1))
                        nc.vector.tensor_scalar_add(
                            dst[:, bass.ts(c, 512)], acc[:], b_sb[t, h][:])

                vp = vp_pool.tile([128, DH + 1, NT], FP16, name=f"vp{h}", tag="vp")
                nc.sync.dma_start(vp[:, DH, :], ones16[:])
                for i in range(NT):
                    v_ps = ps_s.tile([128, DH], F32, name=f"vps{h}{i}", tag="s")
                    nc.tensor.transpose(v_ps[:], vT[:, bass.ts(i, 128)],
                                        id_sb[0:DH, 0:DH])
                    nc.vector.tensor_copy(vp[:, 0:DH, i], v_ps[:])
                st[h].update(qT=qT, kT=kT, vp=vp)

            def emit_B(h):
                """Causal attention k-loop + z' copy + reciprocal columns."""
                qT, kT, vp = st[h]["qT"], st[h]["kT"], st[h]["vp"]
                zT = zt_pool.tile([DH + 1, S], F32R, name=f"zT{h}", tag="zT")
                srow = sr_pool.tile([DH + 1, S], F32, name=f"srow{h}", tag="srow")
                rc = rc_pool.tile([128, NT], F32, name=f"rc{h}", tag="rc")
                for c in range(4):
                    z_ps = ps_z.tile([DH + 1, 512], F32, name=f"z{h}{c}", tag="z")
                    for i in range(4 * c + 4):
                        qlo = max(512 * c, 128 * i)
                        w = 512 * (c + 1) - qlo
                        s_ps = ps_s.tile([128, 512], F32, name=f"s{h}{i}{c}", tag="s")
                        nc.tensor.matmul(s_ps[:, 0:w], kT[:, bass.ts(i, 128)],
                                         qT[:, qlo:qlo + w], start=True, stop=True)
                        P = p_pool.tile([128, 512], FP16, name=f"P{h}{i}{c}", tag="P")
                        nc.scalar.activation(P[:, 0:w], s_ps[:, 0:w],
                                             AF.Exp, scale=0.125)
                        if qlo == 128 * i:
                            nc.vector.tensor_mul(P[:, 0:128], P[:, 0:128], mask_sb[:])
                        nc.tensor.matmul(
                            z_ps[:, qlo - 512 * c: qlo - 512 * c + w],
                            vp[:, :, i], P[:, 0:w],
                            start=(i == 0), stop=(i == 4 * c + 3))
                    nc.vector.tensor_copy(zT[:, bass.ts(c, 512)], z_ps[:])
                    nc.vector.tensor_copy(srow[DH:DH + 1, bass.ts(c, 512)],
                                          z_ps[DH:DH + 1, :])
                    for j in range(4 * c, 4 * c + 4):
                        rc_ps = ps_s.tile([128, 1], F32, name=f"rcp{h}{j}", tag="s")
                        nc.tensor.transpose(rc_ps[:], srow[DH:DH + 1, bass.ts(j, 128)],
                                            id_sb[DH:DH + 1, DH:DH + 1])
                        nc.vector.reciprocal(rc[:, j:j + 1], rc_ps[:])
                st[h].update(zT=zT, rc=rc)

            def emit_C(h):
                """Output projection + per-row softmax scale + store."""
                zT, rc = st[h]["zT"], st[h]["rc"]
                for quarter in range(4):
                    ob = out_pool.tile([128, 4, DM], F32, name=f"ob{h}{quarter}",
                                       tag="ob")
                    for a in range(4):
                        j = 4 * quarter + a
                        for (mo, mw) in ((0, 512), (512, 256)):
                            a_ps = ps_a.tile([128, 512], F32,
                                             name=f"a{h}{j}{mo}", tag="a")
                            nc.tensor.matmul(a_ps[:, 0:mw],
                                             zT[:, bass.ts(j, 128)],
                                             wo_sb[h][:, mo:mo + mw],
                                             start=True, stop=True)
                            nc.vector.tensor_scalar_mul(ob[:, a, mo:mo + mw],
                                                        a_ps[:, 0:mw],
                                                        rc[:, j:j + 1])
                    nc.gpsimd.dma_start(
                        out[h, bass.ts(quarter, 512), :]
                           .rearrange("(a p) m -> p a m", p=128),
                        ob[:])

            # software-pipelined emission: A0 B0 A1 C0 B1 A2 C1 B2 C2
            emit_A(0)
            emit_B(0)
            emit_A(1)
            emit_C(0)
            emit_B(1)
            emit_A(2)
            emit_C(1)
            emit_B(2)
            emit_C(2)
    nc.compile()
    return nc


_CACHED = None


def _program():
    global _CACHED
    if _CACHED is None:
        _CACHED = build_program()
    return _CACHED


def _make_in_maps(inputs):
    xq_f = np.asarray(inputs["normalized_resid_pre_q"], dtype=np.float32)
    xk_f = np.asarray(inputs["normalized_resid_pre_k"], dtype=np.float32)
    xv_f = np.asarray(inputs["normalized_resid_pre_v"], dtype=np.float32)
    WQ = np.asarray(inputs["W_Q"], dtype=np.float32)
    WK = np.asarray(inputs["W_K"], dtype=np.float32)
    WV = np.asarray(inputs["W_V"], dtype=np.float32)
    WO = np.asarray(inputs["W_O"], dtype=np.float32)
    bQ = np.asarray(inputs["b_Q"], dtype=np.float32)
    bK = np.asarray(inputs["b_K"], dtype=np.float32)
    bV = np.asarray(inputs["b_V"], dtype=np.float32)
    bO = np.asarray(inputs["b_O"], dtype=np.float32)

    import ml_dtypes
    ident = np.eye(128, dtype=np.float32)
    maskd = (np.arange(128)[:, None] <= np.arange(128)[None, :]).astype(np.float16)
    ones16 = np.ones((128, NT), np.float16)

    in_maps = []
    for c in range(N_CORES):
        b = c % 2
        hg = c // 2
        hs = slice(HPC * hg, HPC * hg + HPC)
        m = {
            "xq": np.ascontiguousarray(xq_f[b, :, hs, :].transpose(1, 2, 0)),
            "xk": np.ascontiguousarray(xk_f[b, :, hs, :].transpose(1, 2, 0)),
            "xv": np.ascontiguousarray(xv_f[b, :, hs, :].transpose(1, 2, 0)),
            "wq": np.ascontiguousarray(WQ[hs].reshape(HPC, MT, 128, DH)),
            "wk": np.ascontiguousarray(WK[hs].reshape(HPC, MT, 128, DH)),
            "wv": np.ascontiguousarray(WV[hs].reshape(HPC, MT, 128, DH)),
            "wo": np.ascontiguousarray(np.concatenate(
                [WO[hs], np.broadcast_to(bO / H, (HPC, 1, DM))], axis=1)),
            "bq": np.ascontiguousarray(bQ[hs].reshape(HPC, DH, 1)),
            "bk": np.ascontiguousarray(bK[hs].reshape(HPC, DH, 1)),
            "bv": np.ascontiguousarray(bV[hs].reshape(HPC, DH, 1)),
            "ident": ident,
            "maskd": maskd,
            "ones16": ones16,
        }
        in_maps.append(m)
    return in_maps


def run(inputs, trace=False, **kw):
    nc = _program()
    in_maps = _make_in_maps(inputs)
    res = run_bass_kernel_spmd(nc, in_maps, core_ids=list(range(N_CORES)),
                               trace=trace, **kw)
    full = np.zeros((B, S, H, DM), np.float32)
    for c in range(N_CORES):
        b = c % 2
        hg = c // 2
        o = res.results[c]["out"]
        for j in range(HPC):
            full[b, :, HPC * hg + j, :] = o[j]
    return full, res


def kernel(**inputs):
    full, _ = run(inputs)
    return full


# revision 5
# speedup vs baseline: 1.0571x; 1.0571x over previous
"""Trainium2 Bass kernel for per-head causal attention (nn_Attention_52896817217709).

Sharding: 8 cores = 4 head-groups (3 heads each) x 2 batches.
Per core, per head h (S=2048, D_MODEL=768, D_HEAD=64):
  qT = W_Q[h].T @ Xq[h].T        (host supplies X pre-transposed: [768, 2048])
  kT, vT analogous
  S^T[k, q] = kT_chunk.T @ qT    (scores transposed: softmax-k on partitions)
  P = exp(0.125 * S^T) in fp16   (masked diagonal tile; strictly-upper tiles skipped)
  z'[d', q] = sum_k v'[k, d'].T @ P[k, q]   with v' = [v | 1] -> row 64 = softmax sums
  attn[q, m] = (z'^T_chunk.T @ [W_O; b_O/H]) * (1/sums[q])
Projections/scores/output matmuls in float32r; probability path in fp16.
The attention loop is chunk-major (one z' accumulator live) with the S^T+exp
stage running LOOKAHEAD iterations ahead of the z' matmuls so the PE never
stalls on the scalar-engine exp.
"""
import sys
import os
import numpy as np

for _p in ("/opt/trn_rl_repo", "/root/.axon_site/_ro/trn_rl_repo"):
    if os.path.isdir(_p) and _p not in sys.path:
        sys.path.insert(0, _p)

import concourse.bass as bass
import concourse.tile as tile
from concourse import bacc, mybir
from concourse.bass_utils import run_bass_kernel_spmd

F32 = mybir.dt.float32
F32R = mybir.dt.float32r
FP16 = mybir.dt.float16
AF = mybir.ActivationFunctionType

B, S, H, DM, DH = 2, 2048, 12, 768, 64
HPC = 3            # heads per core
NT = S // 128      # 16 s-tiles
MT = DM // 128     # 6 m-tiles
N_CORES = 8
LOOKAHEAD = 3      # S^T/exp stages in flight ahead of z'


def build_program():
    nc = bacc.Bacc("TRN2", target_bir_lowering=False, debug=False)

    xq = nc.dram_tensor("xq", [HPC, DM, S], F32R, kind="ExternalInput")
    xk = nc.dram_tensor("xk", [HPC, DM, S], F32R, kind="ExternalInput")
    xv = nc.dram_tensor("xv", [HPC, DM, S], F32R, kind="ExternalInput")
    wq = nc.dram_tensor("wq", [HPC, MT, 128, DH], F32R, kind="ExternalInput")
    wk = nc.dram_tensor("wk", [HPC, MT, 128, DH], F32R, kind="ExternalInput")
    wv = nc.dram_tensor("wv", [HPC, MT, 128, DH], F32R, kind="ExternalInput")
    wo = nc.dram_tensor("wo", [HPC, DH + 1, DM], F32R, kind="ExternalInput")
    bq = nc.dram_tensor("bq", [HPC, DH, 1], F32, kind="ExternalInput")
    bk = nc.dram_tensor("bk", [HPC, DH, 1], F32, kind="ExternalInput")
    bv = nc.dram_tensor("bv", [HPC, DH, 1], F32, kind="ExternalInput")
    ident = nc.dram_tensor("ident", [128, 128], F32, kind="ExternalInput")
    maskd = nc.dram_tensor("maskd", [128, 128], FP16, kind="ExternalInput")
    ones16 = nc.dram_tensor("ones16", [128, NT], FP16, kind="ExternalInput")
    out = nc.dram_tensor("out", [HPC, S, DM], F32, kind="ExternalOutput")

    with tile.TileContext(nc) as tc:
        with (
            tc.tile_pool(name="wpool", bufs=1) as wpool,
            tc.tile_pool(name="xt", bufs=3) as xt_pool,
            tc.tile_pool(name="qk", bufs=2) as qk_pool,
            tc.tile_pool(name="vtp", bufs=1) as vt_pool,
            tc.tile_pool(name="vp", bufs=2) as vp_pool,
            tc.tile_pool(name="pp", bufs=4) as p_pool,
            tc.tile_pool(name="zt", bufs=1) as zt_pool,
            tc.tile_pool(name="sr", bufs=1) as sr_pool,
            tc.tile_pool(name="rc", bufs=2) as rc_pool,
            tc.tile_pool(name="ob", bufs=2) as out_pool,
            tc.tile_pool(name="ps_s", bufs=4, space="PSUM") as ps_s,
            tc.tile_pool(name="ps_a", bufs=2, space="PSUM") as ps_a,
            tc.tile_pool(name="ps_z", bufs=2, space="PSUM") as ps_z,
        ):
            id_sb = wpool.tile([128, 128], F32, name="id_sb")
            nc.sync.dma_start(id_sb[:], ident[:])
            mask_sb = wpool.tile([128, 128], FP16, name="mask_sb")
            nc.sync.dma_start(mask_sb[:], maskd[:])

            st = [dict() for _ in range(HPC)]   # per-head live tiles

            def emit_A(h):
                """Loads + projections + v' build for head h."""
                qT = qk_pool.tile([DH, S], F32R, name=f"qT{h}", tag="qT")
                kT = qk_pool.tile([DH, S], F32R, name=f"kT{h}", tag="kT")
                vT = vt_pool.tile([DH, S], F32, name=f"vT{h}", tag="vT")
                for t, xd, wd, bd, dst in (("q", xq, wq, bq, qT),
                                           ("k", xk, wk, bk, kT),
                                           ("v", xv, wv, bv, vT)):
                    quarters = []
                    for n in range(4):
                        xh = xt_pool.tile([128, MT, 512], F32R,
                                          name=f"x{t}{h}_{n}", tag="xt")
                        nc.sync.dma_start(
                            xh[:],
                            xd[h].rearrange("(a p) s -> p a s", p=128)
                                 [:, :, bass.ts(n, 512)])
                        quarters.append(xh)
                    wt = wpool.tile([128, MT, DH], F32R, name=f"w{t}{h}")
                    nc.sync.dma_start(wt[:], wd[h].rearrange("a p d -> p a d"))
                    bt = wpool.tile([DH, 1], F32, name=f"b{t}{h}")
                    nc.sync.dma_start(bt[:], bd[h])
                    for c in range(4):
                        acc = ps_s.tile([DH, 512], F32, name=f"acc{t}{h}{c}", tag="s")
                        for mt in range(MT):
                            nc.tensor.matmul(
                                acc[:], wt[:, mt, :],
                                quarters[c][:, mt, :],
                                start=(mt == 0), stop=(mt == MT - 1))
                        nc.vector.tensor_scalar_add(
                            dst[:, bass.ts(c, 512)], acc[:], bt[:])

                vp = vp_pool.tile([128, DH + 1, NT], FP16, name=f"vp{h}", tag="vp")
                nc.sync.dma_start(vp[:, DH, :], ones16[:])
                for i in range(NT):
                    v_ps = ps_s.tile([128, DH], F32, name=f"vps{h}{i}", tag="s")
                    nc.tensor.transpose(v_ps[:], vT[:, bass.ts(i, 128)],
                                        id_sb[0:DH, 0:DH])
                    nc.vector.tensor_copy(vp[:, 0:DH, i], v_ps[:])
                st[h].update(qT=qT, kT=kT, vp=vp)

            def emit_B(h):
                """Causal attention: chunk-major with S^T lookahead."""
                qT, kT, vp = st[h]["qT"], st[h]["kT"], st[h]["vp"]
                zT = zt_pool.tile([DH + 1, S], F32R, name=f"zT{h}", tag="zT")
                srow = sr_pool.tile([DH + 1, S], F32, name=f"srow{h}", tag="srow")
                rc = rc_pool.tile([128, NT], F32, name=f"rc{h}", tag="rc")
                wot = wpool.tile([DH + 1, DM], F32R, name=f"wo{h}")
                nc.sync.dma_start(wot[:], wo[h])
                st[h]["wo"] = wot

                def emit_rc(c):
                    for j in range(4 * c, 4 * c + 4):
                        rc_ps = ps_s.tile([128, 1], F32, name=f"rcp{h}{j}", tag="s")
                        nc.tensor.transpose(
                            rc_ps[:], srow[DH:DH + 1, bass.ts(j, 128)],
                            id_sb[DH:DH + 1, DH:DH + 1])
                        nc.vector.reciprocal(rc[:, j:j + 1], rc_ps[:])

                for c in range(4):
                    z_ps = ps_z.tile([DH + 1, 512], F32, name=f"z{h}{c}", tag="z")
                    n_i = 4 * c + 4

                    def emit_S(i):
                        qlo = max(512 * c, 128 * i)
                        w = 512 * (c + 1) - qlo
                        s_ps = ps_s.tile([128, 512], F32,
                                         name=f"s{h}{c}{i}", tag="s")
                        nc.tensor.matmul(s_ps[:, 0:w], kT[:, bass.ts(i, 128)],
                                         qT[:, qlo:qlo + w], start=True, stop=True)
                        P = p_pool.tile([128, 512], FP16,
                                        name=f"P{h}{c}{i}", tag="P")
                        nc.scalar.activation(P[:, 0:w], s_ps[:, 0:w],
                                             AF.Exp, scale=0.125)
                        if qlo == 128 * i:
                            nc.vector.tensor_mul(P[:, 0:128], P[:, 0:128],
                                                 mask_sb[:])
                        return (qlo, w, P)

                    staged = [emit_S(i) for i in range(min(LOOKAHEAD, n_i))]
                    if c > 0:
                        emit_rc(c - 1)   # previous chunk's recip columns
                    for i in range(n_i):
                        if i + LOOKAHEAD < n_i:
                            staged.append(emit_S(i + LOOKAHEAD))
                        qlo, w, P = staged[i]
                        nc.tensor.matmul(
                            z_ps[:, qlo - 512 * c: qlo - 512 * c + w],
                            vp[:, :, i], P[:, 0:w],
                            start=(i == 0), stop=(i == n_i - 1))
                    nc.vector.tensor_copy(zT[:, bass.ts(c, 512)], z_ps[:])
                    nc.vector.tensor_copy(srow[DH:DH + 1, bass.ts(c, 512)],
                                          z_ps[DH:DH + 1, :])
                emit_rc(3)
                st[h].update(zT=zT, rc=rc)

            def emit_C(h):
                """Output projection + per-row softmax scale + store."""
                zT, rc, wot = st[h]["zT"], st[h]["rc"], st[h]["wo"]
                for quarter in range(4):
                    ob = out_pool.tile([128, 4, DM], F32, name=f"ob{h}{quarter}",
                                       tag="ob")
                    for a in range(4):
                        j = 4 * quarter + a
                        for (mo, mw) in ((0, 512), (512, 256)):
                            a_ps = ps_a.tile([128, 512], F32,
                                             name=f"a{h}{j}{mo}", tag="a")
                            nc.tensor.matmul(a_ps[:, 0:mw],
                                             zT[:, bass.ts(j, 128)],
                                             wot[:, mo:mo + mw],
                                             start=True, stop=True)
                            if mo == 0:
                                nc.scalar.activation(ob[:, a, mo:mo + mw],
                                                     a_ps[:, 0:mw], AF.Copy,
                                                     scale=rc[:, j:j + 1])
                            else:
                                nc.vector.tensor_scalar_mul(ob[:, a, mo:mo + mw],
                                                            a_ps[:, 0:mw],
                                                            rc[:, j:j + 1])
                    nc.gpsimd.dma_start(
                        out[h, bass.ts(quarter, 512), :]
                           .rearrange("(a p) m -> p a m", p=128),
                        ob[:])

            for h in range(HPC):
                emit_A(h)
                emit_B(h)
                emit_C(h)
    nc.compile()
    return nc


_CACHED = None


def _program():
    global _CACHED
    if _CACHED is None:
        _CACHED = build_program()
    return _CACHED


def _make_in_maps(inputs):
    xq_f = np.asarray(inputs["normalized_resid_pre_q"], dtype=np.float32)
    xk_f = np.asarray(inputs["normalized_resid_pre_k"], dtype=np.float32)
    xv_f = np.asarray(inputs["normalized_resid_pre_v"], dtype=np.float32)
    WQ = np.asarray(inputs["W_Q"], dtype=np.float32)
    WK = np.asarray(inputs["W_K"], dtype=np.float32)
    WV = np.asarray(inputs["W_V"], dtype=np.float32)
    WO = np.asarray(inputs["W_O"], dtype=np.float32)
    bQ = np.asarray(inputs["b_Q"], dtype=np.float32)
    bK = np.asarray(inputs["b_K"], dtype=np.float32)
    bV = np.asarray(inputs["b_V"], dtype=np.float32)
    bO = np.asarray(inputs["b_O"], dtype=np.float32)

    ident = np.eye(128, dtype=np.float32)
    maskd = (np.arange(128)[:, None] <= np.arange(128)[None, :]).astype(np.float16)
    ones16 = np.ones((128, NT), np.float16)

    in_maps = []
    for c in range(N_CORES):
        b = c % 2
        hg = c // 2
        hs = slice(HPC * hg, HPC * hg + HPC)
        m = {
            "xq": np.ascontiguousarray(xq_f[b, :, hs, :].transpose(1, 2, 0)),
            "xk": np.ascontiguousarray(xk_f[b, :, hs, :].transpose(1, 2, 0)),
            "xv": np.ascontiguousarray(xv_f[b, :, hs, :].transpose(1, 2, 0)),
            "wq": np.ascontiguousarray(WQ[hs].reshape(HPC, MT, 128, DH)),
            "wk": np.ascontiguousarray(WK[hs].reshape(HPC, MT, 128, DH)),
            "wv": np.ascontiguousarray(WV[hs].reshape(HPC, MT, 128, DH)),
            "wo": np.ascontiguousarray(np.concatenate(
                [WO[hs], np.broadcast_to(bO / H, (HPC, 1, DM))], axis=1)),
            "bq": np.ascontiguousarray(bQ[hs].reshape(HPC, DH, 1)),
            "bk": np.ascontiguousarray(bK[hs].reshape(HPC, DH, 1)),
            "bv": np.ascontiguousarray(bV[hs].reshape(HPC, DH, 1)),
            "ident": ident,
            "maskd": maskd,
            "ones16": ones16,
        }
        in_maps.append(m)
    return in_maps


def run(inputs, trace=False, **kw):
    nc = _program()
    in_maps = _make_in_maps(inputs)
    res = run_bass_kernel_spmd(nc, in_maps, core_ids=list(range(N_CORES)),
                               trace=trace, **kw)
    full = np.zeros((B, S, H, DM), np.float32)
    for c in range(N_CORES):
        b = c % 2
        hg = c // 2
        o = res.results[c]["out"]
        for j in range(HPC):
            full[b, :, HPC * hg + j, :] = o[j]
    return full, res


def kernel(**inputs):
    full, _ = run(inputs)
    return full


# revision 6
# speedup vs baseline: 1.1994x; 1.1346x over previous
"""Trainium2 Bass kernel for per-head causal attention (nn_Attention_52896817217709).

Sharding: 8 cores = 4 head-groups (3 heads each) x 2 batches.
Per core, per head h (S=2048, D_MODEL=768, D_HEAD=64):
  qT = W_Q[h].T @ Xq[h].T        (host supplies X pre-transposed: [768, 2048])
  kT, vT analogous
  S^T[k, q] = kT_chunk.T @ qT    (scores transposed: softmax-k on partitions)
  P = exp(0.125 * S^T) in fp16   (masked diagonal tile; strictly-upper tiles skipped)
  z'[d', q] = sum_k v'[k, d'].T @ P[k, q]   with v' = [v | 1] -> row 64 = softmax sums
  attn[q, m] = (z'^T_chunk.T @ [W_O; b_O/H]) * (1/sums[q])
Projections/scores/output matmuls in float32r; probability path in fp16.
The attention loop is chunk-major (one z' accumulator live) with the S^T+exp
stage running LOOKAHEAD iterations ahead of the z' matmuls so the PE never
stalls on the scalar-engine exp.
"""
import sys
import os
import numpy as np

for _p in ("/opt/trn_rl_repo", "/root/.axon_site/_ro/trn_rl_repo"):
    if os.path.isdir(_p) and _p not in sys.path:
        sys.path.insert(0, _p)

import concourse.bass as bass
import concourse.tile as tile
from concourse import bacc, mybir
from concourse.bass_utils import run_bass_kernel_spmd

F32 = mybir.dt.float32
F32R = mybir.dt.float32r
FP16 = mybir.dt.float16
AF = mybir.ActivationFunctionType

B, S, H, DM, DH = 2, 2048, 12, 768, 64
HPC = 3            # heads per core
NT = S // 128      # 16 s-tiles
MT = DM // 128     # 6 m-tiles
N_CORES = 8
LOOKAHEAD = 3      # S^T/exp stages in flight ahead of z'


def build_program():
    nc = bacc.Bacc("TRN2", target_bir_lowering=False, debug=False)

    xq = nc.dram_tensor("xq", [HPC, DM, S], F32R, kind="ExternalInput")
    xk = nc.dram_tensor("xk", [HPC, DM, S], F32R, kind="ExternalInput")
    xv = nc.dram_tensor("xv", [HPC, DM, S], F32R, kind="ExternalInput")
    wq = nc.dram_tensor("wq", [HPC, MT, 128, DH], F32R, kind="ExternalInput")
    wk = nc.dram_tensor("wk", [HPC, MT, 128, DH], F32R, kind="ExternalInput")
    wv = nc.dram_tensor("wv", [HPC, MT, 128, DH], F32R, kind="ExternalInput")
    wo = nc.dram_tensor("wo", [HPC, DH + 1, DM], F32R, kind="ExternalInput")
    bq = nc.dram_tensor("bq", [HPC, DH, 1], F32, kind="ExternalInput")
    bk = nc.dram_tensor("bk", [HPC, DH, 1], F32, kind="ExternalInput")
    bv = nc.dram_tensor("bv", [HPC, DH, 1], F32, kind="ExternalInput")
    ident = nc.dram_tensor("ident", [128, 128], F32, kind="ExternalInput")
    maskd = nc.dram_tensor("maskd", [128, 128], FP16, kind="ExternalInput")
    ones16 = nc.dram_tensor("ones16", [128, NT], FP16, kind="ExternalInput")
    out = nc.dram_tensor("out", [HPC, S, DM], F32, kind="ExternalOutput")

    with tile.TileContext(nc) as tc:
        with (
            tc.tile_pool(name="wpool", bufs=1) as wpool,
            tc.tile_pool(name="xt", bufs=2) as xt_pool,
            tc.tile_pool(name="qk", bufs=2) as qk_pool,
            tc.tile_pool(name="vtp", bufs=1) as vt_pool,
            tc.tile_pool(name="vp", bufs=2) as vp_pool,
            tc.tile_pool(name="pp", bufs=4) as p_pool,
            tc.tile_pool(name="zt", bufs=1) as zt_pool,
            tc.tile_pool(name="sr", bufs=1) as sr_pool,
            tc.tile_pool(name="rc", bufs=2) as rc_pool,
            tc.tile_pool(name="ob", bufs=2) as out_pool,
            tc.tile_pool(name="ps_s", bufs=4, space="PSUM") as ps_s,
            tc.tile_pool(name="ps_a", bufs=2, space="PSUM") as ps_a,
            tc.tile_pool(name="ps_z", bufs=2, space="PSUM") as ps_z,
        ):
            id_sb = wpool.tile([128, 128], F32, name="id_sb")
            nc.sync.dma_start(id_sb[:], ident[:])
            mask_sb = wpool.tile([128, 128], FP16, name="mask_sb")
            nc.sync.dma_start(mask_sb[:], maskd[:])

            st = [dict() for _ in range(HPC)]   # per-head live tiles

            def emit_A(h):
                """Loads + projections + v' build for head h."""
                qT = qk_pool.tile([DH, S], F32R, name=f"qT{h}", tag="qT")
                kT = qk_pool.tile([DH, S], F32R, name=f"kT{h}", tag="kT")
                vT = vt_pool.tile([DH, S], F32, name=f"vT{h}", tag="vT")
                for t, xd, wd, bd, dst in (("q", xq, wq, bq, qT),
                                           ("k", xk, wk, bk, kT),
                                           ("v", xv, wv, bv, vT)):
                    xh = xt_pool.tile([128, MT, S], F32R,
                                      name=f"x{t}{h}", tag="xt")
                    nc.sync.dma_start(
                        xh[:], xd[h].rearrange("(a p) s -> p a s", p=128))
                    wt = wpool.tile([128, MT, DH], F32R, name=f"w{t}{h}")
                    nc.sync.dma_start(wt[:], wd[h].rearrange("a p d -> p a d"))
                    bt = wpool.tile([DH, 1], F32, name=f"b{t}{h}")
                    nc.sync.dma_start(bt[:], bd[h])
                    for c in range(4):
                        acc = ps_s.tile([DH, 512], F32, name=f"acc{t}{h}{c}", tag="s")
                        for mt in range(MT):
                            nc.tensor.matmul(
                                acc[:], wt[:, mt, :],
                                xh[:, mt, bass.ts(c, 512)],
                                start=(mt == 0), stop=(mt == MT - 1))
                        nc.vector.tensor_scalar_add(
                            dst[:, bass.ts(c, 512)], acc[:], bt[:])

                vp = vp_pool.tile([128, DH + 1, NT], FP16, name=f"vp{h}", tag="vp")
                nc.sync.dma_start(vp[:, DH, :], ones16[:])
                for i in range(NT):
                    v_ps = ps_s.tile([128, DH], F32, name=f"vps{h}{i}", tag="s")
                    nc.tensor.transpose(v_ps[:], vT[:, bass.ts(i, 128)],
                                        id_sb[0:DH, 0:DH])
                    nc.vector.tensor_copy(vp[:, 0:DH, i], v_ps[:])
                st[h].update(qT=qT, kT=kT, vp=vp)

            def emit_B(h):
                """Causal attention: chunk-major with S^T lookahead."""
                qT, kT, vp = st[h]["qT"], st[h]["kT"], st[h]["vp"]
                zT = zt_pool.tile([DH + 1, S], F32R, name=f"zT{h}", tag="zT")
                srow = sr_pool.tile([DH + 1, S], F32, name=f"srow{h}", tag="srow")
                rc = rc_pool.tile([128, NT], F32, name=f"rc{h}", tag="rc")
                wot = wpool.tile([DH + 1, DM], F32R, name=f"wo{h}")
                nc.sync.dma_start(wot[:], wo[h])
                st[h]["wo"] = wot

                def emit_rc(c):
                    for j in range(4 * c, 4 * c + 4):
                        rc_ps = ps_s.tile([128, 1], F32, name=f"rcp{h}{j}", tag="s")
                        nc.tensor.transpose(
                            rc_ps[:], srow[DH:DH + 1, bass.ts(j, 128)],
                            id_sb[DH:DH + 1, DH:DH + 1])
                        nc.vector.reciprocal(rc[:, j:j + 1], rc_ps[:])

                for c in range(4):
                    z_ps = ps_z.tile([DH + 1, 512], F32, name=f"z{h}{c}", tag="z")
                    n_i = 4 * c + 4

                    def emit_S(i):
                        qlo = max(512 * c, 128 * i)
                        w = 512 * (c + 1) - qlo
                        s_ps = ps_s.tile([128, 512], F32,
                                         name=f"s{h}{c}{i}", tag="s")
                        nc.tensor.matmul(s_ps[:, 0:w], kT[:, bass.ts(i, 128)],
                                         qT[:, qlo:qlo + w], start=True, stop=True)
                        P = p_pool.tile([128, 512], FP16,
                                        name=f"P{h}{c}{i}", tag="P")
                        nc.scalar.activation(P[:, 0:w], s_ps[:, 0:w],
                                             AF.Exp, scale=0.125)
                        if qlo == 128 * i:
                            nc.vector.tensor_mul(P[:, 0:128], P[:, 0:128],
                                                 mask_sb[:])
                        return (qlo, w, P)

                    staged = [emit_S(i) for i in range(min(LOOKAHEAD, n_i))]
                    if c > 0:
                        emit_rc(c - 1)   # previous chunk's recip columns
                    for i in range(n_i):
                        if i + LOOKAHEAD < n_i:
                            staged.append(emit_S(i + LOOKAHEAD))
                        qlo, w, P = staged[i]
                        nc.tensor.matmul(
                            z_ps[:, qlo - 512 * c: qlo - 512 * c + w],
                            vp[:, :, i], P[:, 0:w],
                            start=(i == 0), stop=(i == n_i - 1))
                    nc.vector.tensor_copy(zT[:, bass.ts(c, 512)], z_ps[:])
                    nc.vector.tensor_copy(srow[DH:DH + 1, bass.ts(c, 512)],
                                          z_ps[DH:DH + 1, :])
                emit_rc(3)
                st[h].update(zT=zT, rc=rc)

            def emit_C(h):
                """Output projection + per-row softmax scale + store."""
                zT, rc, wot = st[h]["zT"], st[h]["rc"], st[h]["wo"]
                for quarter in range(4):
                    ob = out_pool.tile([128, 4, DM], F32, name=f"ob{h}{quarter}",
                                       tag="ob")
                    for a in range(4):
                        j = 4 * quarter + a
                        for (mo, mw) in ((0, 512), (512, 256)):
                            a_ps = ps_a.tile([128, 512], F32,
                                             name=f"a{h}{j}{mo}", tag="a")
                            nc.tensor.matmul(a_ps[:, 0:mw],
                                             zT[:, bass.ts(j, 128)],
                                             wot[:, mo:mo + mw],
                                             start=True, stop=True)
                            if mo == 0:
                                nc.scalar.activation(ob[:, a, mo:mo + mw],
                                                     a_ps[:, 0:mw], AF.Copy,
                                                     scale=rc[:, j:j + 1])
                            else:
                                nc.vector.tensor_scalar_mul(ob[:, a, mo:mo + mw],
                                                            a_ps[:, 0:mw],
                                                            rc[:, j:j + 1])
                    nc.gpsimd.dma_start(
                        out[h, bass.ts(quarter, 512), :]
                           .rearrange("(a p) m -> p a m", p=128),
                        ob[:])

            for h in range(HPC):
                emit_A(h)
                emit_B(h)
                emit_C(h)
    nc.compile()
    return nc


_CACHED = None


def _program():
    global _CACHED
    if _CACHED is None:
        _CACHED = build_program()
    return _CACHED


def _make_in_maps(inputs):
    xq_f = np.asarray(inputs["normalized_resid_pre_q"], dtype=np.float32)
    xk_f = np.asarray(inputs["normalized_resid_pre_k"], dtype=np.float32)
    xv_f = np.asarray(inputs["normalized_resid_pre_v"], dtype=np.float32)
    WQ = np.asarray(inputs["W_Q"], dtype=np.float32)
    WK = np.asarray(inputs["W_K"], dtype=np.float32)
    WV = np.asarray(inputs["W_V"], dtype=np.float32)
    WO = np.asarray(inputs["W_O"], dtype=np.float32)
    bQ = np.asarray(inputs["b_Q"], dtype=np.float32)
    bK = np.asarray(inputs["b_K"], dtype=np.float32)
    bV = np.asarray(inputs["b_V"], dtype=np.float32)
    bO = np.asarray(inputs["b_O"], dtype=np.float32)

    ident = np.eye(128, dtype=np.float32)
    maskd = (np.arange(128)[:, None] <= np.arange(128)[None, :]).astype(np.float16)
    ones16 = np.ones((128, NT), np.float16)

    in_maps = []
    for c in range(N_CORES):
        b = c % 2
        hg = c // 2
        hs = slice(HPC * hg, HPC * hg + HPC)
        m = {
            "xq": np.ascontiguousarray(xq_f[b, :, hs, :].transpose(1, 2, 0)),
            "xk": np.ascontiguousarray(xk_f[b, :, hs, :].transpose(1, 2, 0)),
            "xv": np.ascontiguousarray(xv_f[b, :, hs, :].transpose(1, 2, 0)),
            "wq": np.ascontiguousarray(WQ[hs].reshape(HPC, MT, 128, DH)),
            "wk": np.ascontiguousarray(WK[hs].reshape(HPC, MT, 128, DH)),
            "wv": np.ascontiguousarray(WV[hs].reshape(HPC, MT, 128, DH)),
            "wo": np.ascontiguousarray(np.concatenate(
                [WO[hs], np.broadcast_to(bO / H, (HPC, 1, DM))], axis=1)),
            "bq": np.ascontiguousarray(bQ[hs].reshape(HPC, DH, 1)),
            "bk": np.ascontiguousarray(bK[hs].reshape(HPC, DH, 1)),
            "bv": np.ascontiguousarray(bV[hs].reshape(HPC, DH, 1)),
            "ident": ident,
            "maskd": maskd,
            "ones16": ones16,
        }
        in_maps.append(m)
    return in_maps


def run(inputs, trace=False, **kw):
    nc = _program()
    in_maps = _make_in_maps(inputs)
    res = run_bass_kernel_spmd(nc, in_maps, core_ids=list(range(N_CORES)),
                               trace=trace, **kw)
    full = np.zeros((B, S, H, DM), np.float32)
    for c in range(N_CORES):
        b = c % 2
        hg = c // 2
        o = res.results[c]["out"]
        for j in range(HPC):
            full[b, :, HPC * hg + j, :] = o[j]
    return full, res


def kernel(**inputs):
    full, _ = run(inputs)
    return full


# revision 7
# speedup vs baseline: 1.2156x; 1.0135x over previous
"""Trainium2 Bass kernel for per-head causal attention (nn_Attention_52896817217709).

Sharding: 8 cores = 4 head-groups (3 heads each) x 2 batches.
Per core, per head h (S=2048, D_MODEL=768, D_HEAD=64):
  qT = W_Q[h].T @ Xq[h].T        (host supplies X pre-transposed: [768, 2048])
  kT, vT analogous
  S^T[k, q] = kT_chunk.T @ qT    (scores transposed: softmax-k on partitions)
  P = exp(0.125 * S^T) in fp16   (masked diagonal tile; strictly-upper tiles skipped)
  z'[d', q] = sum_k v'[k, d'].T @ P[k, q]   with v' = [v | 1] -> row 64 = softmax sums
  attn[q, m] = (z'^T_chunk.T @ [W_O; b_O/H]) * (1/sums[q])
Projections/scores/output matmuls in float32r; probability path in fp16.
The attention loop is chunk-major (one z' accumulator live) with the S^T+exp
stage running LOOKAHEAD iterations ahead of the z' matmuls so the PE never
stalls on the scalar-engine exp.
"""
import sys
import os
import numpy as np

for _p in ("/opt/trn_rl_repo", "/root/.axon_site/_ro/trn_rl_repo"):
    if os.path.isdir(_p) and _p not in sys.path:
        sys.path.insert(0, _p)

import concourse.bass as bass
import concourse.tile as tile
from concourse import bacc, mybir
from concourse.bass_utils import run_bass_kernel_spmd

F32 = mybir.dt.float32
F32R = mybir.dt.float32r
FP16 = mybir.dt.float16
AF = mybir.ActivationFunctionType

B, S, H, DM, DH = 2, 2048, 12, 768, 64
HPC = 3            # heads per core
NT = S // 128      # 16 s-tiles
MT = DM // 128     # 6 m-tiles
N_CORES = 8
LOOKAHEAD = 3      # S^T/exp stages in flight ahead of z'


def build_program():
    nc = bacc.Bacc("TRN2", target_bir_lowering=False, debug=False)

    xq = nc.dram_tensor("xq", [HPC, DM, S], F32R, kind="ExternalInput")
    xk = nc.dram_tensor("xk", [HPC, DM, S], F32R, kind="ExternalInput")
    xv = nc.dram_tensor("xv", [HPC, DM, S], F32R, kind="ExternalInput")
    wq = nc.dram_tensor("wq", [HPC, MT, 128, DH], F32R, kind="ExternalInput")
    wk = nc.dram_tensor("wk", [HPC, MT, 128, DH], F32R, kind="ExternalInput")
    wv = nc.dram_tensor("wv", [HPC, MT, 128, DH], F32R, kind="ExternalInput")
    wo = nc.dram_tensor("wo", [HPC, DH + 1, DM], F32R, kind="ExternalInput")
    bq = nc.dram_tensor("bq", [HPC, DH, 1], F32, kind="ExternalInput")
    bk = nc.dram_tensor("bk", [HPC, DH, 1], F32, kind="ExternalInput")
    bv = nc.dram_tensor("bv", [HPC, DH, 1], F32, kind="ExternalInput")
    ident = nc.dram_tensor("ident", [128, 128], F32, kind="ExternalInput")
    maskd = nc.dram_tensor("maskd", [128, 128], FP16, kind="ExternalInput")
    ones16 = nc.dram_tensor("ones16", [128, NT], FP16, kind="ExternalInput")
    out = nc.dram_tensor("out", [HPC, S, DM], F32, kind="ExternalOutput")

    with tile.TileContext(nc) as tc:
        with (
            tc.tile_pool(name="wpool", bufs=1) as wpool,
            tc.tile_pool(name="xt", bufs=2) as xt_pool,
            tc.tile_pool(name="qk", bufs=2) as qk_pool,
            tc.tile_pool(name="vtp", bufs=1) as vt_pool,
            tc.tile_pool(name="vp", bufs=2) as vp_pool,
            tc.tile_pool(name="pp", bufs=4) as p_pool,
            tc.tile_pool(name="zt", bufs=1) as zt_pool,
            tc.tile_pool(name="sr", bufs=1) as sr_pool,
            tc.tile_pool(name="rc", bufs=2) as rc_pool,
            tc.tile_pool(name="ob", bufs=2) as out_pool,
            tc.tile_pool(name="ps_s", bufs=4, space="PSUM") as ps_s,
            tc.tile_pool(name="ps_a", bufs=2, space="PSUM") as ps_a,
            tc.tile_pool(name="ps_z", bufs=2, space="PSUM") as ps_z,
        ):
            id_sb = wpool.tile([128, 128], F32, name="id_sb")
            nc.scalar.dma_start(id_sb[:], ident[:])
            mask_sb = wpool.tile([128, 128], FP16, name="mask_sb")
            nc.scalar.dma_start(mask_sb[:], maskd[:])

            st = [dict() for _ in range(HPC)]   # per-head live tiles

            TENS = {"q": (xq, wq, bq), "k": (xk, wk, bk), "v": (xv, wv, bv)}

            def emit_loads(h):
                """Big X loads on the (pure) sync ring; weights on scalar ring."""
                for t in ("q", "k", "v"):
                    xd, wd, bd = TENS[t]
                    xh = xt_pool.tile([128, MT, S], F32R,
                                      name=f"x{t}{h}", tag="xt")
                    nc.sync.dma_start(
                        xh[:], xd[h].rearrange("(a p) s -> p a s", p=128))
                    wt = wpool.tile([128, MT, DH], F32R, name=f"w{t}{h}")
                    nc.scalar.dma_start(wt[:], wd[h].rearrange("a p d -> p a d"))
                    bt = wpool.tile([DH, 1], F32, name=f"b{t}{h}")
                    nc.scalar.dma_start(bt[:], bd[h])
                    st[h][f"x{t}"] = xh
                    st[h][f"w{t}"] = wt
                    st[h][f"b{t}"] = bt

            def emit_proj(h, t):
                """One tensor's projection chains; frees its X slot at the end."""
                if t == "q":
                    dst = qk_pool.tile([DH, S], F32R, name=f"qT{h}", tag="qT")
                    st[h]["qT"] = dst
                elif t == "k":
                    dst = qk_pool.tile([DH, S], F32R, name=f"kT{h}", tag="kT")
                    st[h]["kT"] = dst
                else:
                    dst = vt_pool.tile([DH, S], F32, name=f"vT{h}", tag="vT")
                    st[h]["vT"] = dst
                xh, wt, bt = st[h][f"x{t}"], st[h][f"w{t}"], st[h][f"b{t}"]
                for c in range(4):
                    acc = ps_s.tile([DH, 512], F32, name=f"acc{t}{h}{c}", tag="s")
                    for mt in range(MT):
                        nc.tensor.matmul(
                            acc[:], wt[:, mt, :],
                            xh[:, mt, bass.ts(c, 512)],
                            start=(mt == 0), stop=(mt == MT - 1))
                    nc.vector.tensor_scalar_add(
                        dst[:, bass.ts(c, 512)], acc[:], bt[:])

            def emit_vp(h):
                vT = st[h]["vT"]
                vp = vp_pool.tile([128, DH + 1, NT], FP16, name=f"vp{h}", tag="vp")
                nc.scalar.dma_start(vp[:, DH, :], ones16[:])
                for i in range(NT):
                    v_ps = ps_s.tile([128, DH], F32, name=f"vps{h}{i}", tag="s")
                    nc.tensor.transpose(v_ps[:], vT[:, bass.ts(i, 128)],
                                        id_sb[0:DH, 0:DH])
                    nc.vector.tensor_copy(vp[:, 0:DH, i], v_ps[:])
                st[h]["vp"] = vp

            def emit_B(h, interleave=None):
                """Causal attention: chunk-major with S^T lookahead.
                interleave[c] (optional) emits next-head work after chunk c."""
                qT, kT, vp = st[h]["qT"], st[h]["kT"], st[h]["vp"]
                zT = zt_pool.tile([DH + 1, S], F32R, name=f"zT{h}", tag="zT")
                srow = sr_pool.tile([DH + 1, S], F32, name=f"srow{h}", tag="srow")
                rc = rc_pool.tile([128, NT], F32, name=f"rc{h}", tag="rc")
                wot = wpool.tile([DH + 1, DM], F32R, name=f"wo{h}")
                nc.scalar.dma_start(wot[:], wo[h])
                st[h]["wo"] = wot

                def emit_rc(c):
                    for j in range(4 * c, 4 * c + 4):
                        rc_ps = ps_s.tile([128, 1], F32, name=f"rcp{h}{j}", tag="s")
                        nc.tensor.transpose(
                            rc_ps[:], srow[DH:DH + 1, bass.ts(j, 128)],
                            id_sb[DH:DH + 1, DH:DH + 1])
                        nc.vector.reciprocal(rc[:, j:j + 1], rc_ps[:])

                for c in range(4):
                    z_ps = ps_z.tile([DH + 1, 512], F32, name=f"z{h}{c}", tag="z")
                    n_i = 4 * c + 4

                    def emit_S(i):
                        qlo = max(512 * c, 128 * i)
                        w = 512 * (c + 1) - qlo
                        s_ps = ps_s.tile([128, 512], F32,
                                         name=f"s{h}{c}{i}", tag="s")
                        nc.tensor.matmul(s_ps[:, 0:w], kT[:, bass.ts(i, 128)],
                                         qT[:, qlo:qlo + w], start=True, stop=True)
                        P = p_pool.tile([128, 512], FP16,
                                        name=f"P{h}{c}{i}", tag="P")
                        nc.scalar.activation(P[:, 0:w], s_ps[:, 0:w],
                                             AF.Exp, scale=0.125)
                        if qlo == 128 * i:
                            nc.vector.tensor_mul(P[:, 0:128], P[:, 0:128],
                                                 mask_sb[:])
                        return (qlo, w, P)

                    staged = [emit_S(i) for i in range(min(LOOKAHEAD, n_i))]
                    if c > 0:
                        emit_rc(c - 1)   # previous chunk's recip columns
                    for i in range(n_i):
                        if i + LOOKAHEAD < n_i:
                            staged.append(emit_S(i + LOOKAHEAD))
                        qlo, w, P = staged[i]
                        nc.tensor.matmul(
                            z_ps[:, qlo - 512 * c: qlo - 512 * c + w],
                            vp[:, :, i], P[:, 0:w],
                            start=(i == 0), stop=(i == n_i - 1))
                    nc.vector.tensor_copy(zT[:, bass.ts(c, 512)], z_ps[:])
                    nc.vector.tensor_copy(srow[DH:DH + 1, bass.ts(c, 512)],
                                          z_ps[DH:DH + 1, :])
                    if interleave and c in interleave:
                        interleave[c]()
                emit_rc(3)
                st[h].update(zT=zT, rc=rc)

            def emit_C(h):
                """Output projection + per-row softmax scale + store."""
                zT, rc, wot = st[h]["zT"], st[h]["rc"], st[h]["wo"]
                for quarter in range(4):
                    ob = out_pool.tile([128, 4, DM], F32, name=f"ob{h}{quarter}",
                                       tag="ob")
                    for a in range(4):
                        j = 4 * quarter + a
                        for (mo, mw) in ((0, 512), (512, 256)):
                            a_ps = ps_a.tile([128, 512], F32,
                                             name=f"a{h}{j}{mo}", tag="a")
                            nc.tensor.matmul(a_ps[:, 0:mw],
                                             zT[:, bass.ts(j, 128)],
                                             wot[:, mo:mo + mw],
                                             start=True, stop=True)
                            if mo == 0:
                                nc.scalar.activation(ob[:, a, mo:mo + mw],
                                                     a_ps[:, 0:mw], AF.Copy,
                                                     scale=rc[:, j:j + 1])
                            else:
                                nc.vector.tensor_scalar_mul(ob[:, a, mo:mo + mw],
                                                            a_ps[:, 0:mw],
                                                            rc[:, j:j + 1])
                    nc.gpsimd.dma_start(
                        out[h, bass.ts(quarter, 512), :]
                           .rearrange("(a p) m -> p a m", p=128),
                        ob[:])

            emit_loads(0)
            for t in ("q", "k", "v"):
                emit_proj(0, t)
            emit_vp(0)
            for h in range(HPC):
                nxt = h + 1
                if nxt < HPC:
                    emit_loads(nxt)
                    inter = {
                        0: lambda n=nxt: emit_proj(n, "q"),
                        1: lambda n=nxt: emit_proj(n, "k"),
                        2: lambda n=nxt: (emit_proj(n, "v"), emit_vp(n)),
                    }
                else:
                    inter = None
                emit_B(h, interleave=inter)
                emit_C(h)
    nc.compile()
    return nc


_CACHED = None


def _program():
    global _CACHED
    if _CACHED is None:
        _CACHED = build_program()
    return _CACHED


def _make_in_maps(inputs):
    xq_f = np.asarray(inputs["normalized_resid_pre_q"], dtype=np.float32)
    xk_f = np.asarray(inputs["normalized_resid_pre_k"], dtype=np.float32)
    xv_f = np.asarray(inputs["normalized_resid_pre_v"], dtype=np.float32)
    WQ = np.asarray(inputs["W_Q"], dtype=np.float32)
    WK = np.asarray(inputs["W_K"], dtype=np.float32)
    WV = np.asarray(inputs["W_V"], dtype=np.float32)
    WO = np.asarray(inputs["W_O"], dtype=np.float32)
    bQ = np.asarray(inputs["b_Q"], dtype=np.float32)
    bK = np.asarray(inputs["b_K"], dtype=np.float32)
    bV = np.asarray(inputs["b_V"], dtype=np.float32)
    bO = np.asarray(inputs["b_O"], dtype=np.float32)

    ident = np.eye(128, dtype=np.float32)
    maskd = (np.arange(128)[:, None] <= np.arange(128)[None, :]).astype(np.float16)
    ones16 = np.ones((128, NT), np.float16)

    in_maps = []
    for c in range(N_CORES):
        b = c % 2
        hg = c // 2
        hs = slice(HPC * hg, HPC * hg + HPC)
        m = {
            "xq": np.ascontiguousarray(xq_f[b, :, hs, :].transpose(1, 2, 0)),
            "xk": np.ascontiguousarray(xk_f[b, :, hs, :].transpose(1, 2, 0)),
            "xv": np.ascontiguousarray(xv_f[b, :, hs, :].transpose(1, 2, 0)),
            "wq": np.ascontiguousarray(WQ[hs].reshape(HPC, MT, 128, DH)),
            "wk": np.ascontiguousarray(WK[hs].reshape(HPC, MT, 128, DH)),
            "wv": np.ascontiguousarray(WV[hs].reshape(HPC, MT, 128, DH)),
            "wo": np.ascontiguousarray(np.concatenate(
                [WO[hs], np.broadcast_to(bO / H, (HPC, 1, DM))], axis=1)),
            "bq": np.ascontiguousarray(bQ[hs].reshape(HPC, DH, 1)),
            "bk": np.ascontiguousarray(bK[hs].reshape(HPC, DH, 1)),
            "bv": np.ascontiguousarray(bV[hs].reshape(HPC, DH, 1)),
            "ident": ident,
            "maskd": maskd,
            "ones16": ones16,
        }
        in_maps.append(m)
    return in_maps


def run(inputs, trace=False, **kw):
    nc = _program()
    in_maps = _make_in_maps(inputs)
    res = run_bass_kernel_spmd(nc, in_maps, core_ids=list(range(N_CORES)),
                               trace=trace, **kw)
    full = np.zeros((B, S, H, DM), np.float32)
    for c in range(N_CORES):
        b = c % 2
        hg = c // 2
        o = res.results[c]["out"]
        for j in range(HPC):
            full[b, :, HPC * hg + j, :] = o[j]
    return full, res


def kernel(**inputs):
    full, _ = run(inputs)
    return full


# revision 8
# speedup vs baseline: 1.2620x; 1.0382x over previous
"""Trainium2 Bass kernel for per-head causal attention (nn_Attention_52896817217709).

Sharding: 8 cores = 4 head-groups (3 heads each) x 2 batches.
Per core, per head h (S=2048, D_MODEL=768, D_HEAD=64):
  qT = W_Q[h].T @ Xq[h].T        (host supplies X pre-transposed: [768, 2048])
  kT, vT analogous
  S^T[k, q] = kT_chunk.T @ qT    (scores transposed: softmax-k on partitions)
  P = exp(0.125 * S^T) in fp16   (masked diagonal tile; strictly-upper tiles skipped)
  z'[d', q] = sum_k v'[k, d'].T @ P[k, q]   with v' = [v | 1] -> row 64 = softmax sums
  attn[q, m] = (z'^T_chunk.T @ [W_O; b_O/H]) * (1/sums[q])
Projections/scores/output matmuls in float32r; probability path in fp16.
The attention loop is chunk-major (one z' accumulator live) with the S^T+exp
stage running LOOKAHEAD iterations ahead of the z' matmuls so the PE never
stalls on the scalar-engine exp.
"""
import sys
import os
import numpy as np

for _p in ("/opt/trn_rl_repo", "/root/.axon_site/_ro/trn_rl_repo"):
    if os.path.isdir(_p) and _p not in sys.path:
        sys.path.insert(0, _p)

import concourse.bass as bass
import concourse.tile as tile
from concourse import bacc, mybir
from concourse.bass_utils import run_bass_kernel_spmd

F32 = mybir.dt.float32
F32R = mybir.dt.float32r
FP16 = mybir.dt.float16
AF = mybir.ActivationFunctionType

B, S, H, DM, DH = 2, 2048, 12, 768, 64
HPC = 3            # heads per core
NT = S // 128      # 16 s-tiles
MT = DM // 128     # 6 m-tiles
N_CORES = 8
LOOKAHEAD = 3      # S^T/exp stages in flight ahead of z'


def build_program():
    nc = bacc.Bacc("TRN2", target_bir_lowering=False, debug=False)

    xq = nc.dram_tensor("xq", [HPC, DM, S], F32R, kind="ExternalInput")
    xk = nc.dram_tensor("xk", [HPC, DM, S], F32R, kind="ExternalInput")
    xv = nc.dram_tensor("xv", [HPC, DM, S], F32R, kind="ExternalInput")
    wq = nc.dram_tensor("wq", [HPC, MT, 128, DH], F32R, kind="ExternalInput")
    wk = nc.dram_tensor("wk", [HPC, MT, 128, DH], F32R, kind="ExternalInput")
    wv = nc.dram_tensor("wv", [HPC, MT, 128, DH], F32R, kind="ExternalInput")
    wo = nc.dram_tensor("wo", [HPC, DH + 1, DM], F32R, kind="ExternalInput")
    bq = nc.dram_tensor("bq", [HPC, DH, 1], F32, kind="ExternalInput")
    bk = nc.dram_tensor("bk", [HPC, DH, 1], F32, kind="ExternalInput")
    bv = nc.dram_tensor("bv", [HPC, DH, 1], F32, kind="ExternalInput")
    ident = nc.dram_tensor("ident", [128, 128], F32, kind="ExternalInput")
    maskd = nc.dram_tensor("maskd", [128, 128], FP16, kind="ExternalInput")
    ones16 = nc.dram_tensor("ones16", [128, NT], FP16, kind="ExternalInput")
    out = nc.dram_tensor("out", [HPC, S, DM], F32, kind="ExternalOutput")

    with tile.TileContext(nc) as tc:
        with (
            tc.tile_pool(name="wpool", bufs=1) as wpool,
            tc.tile_pool(name="xt", bufs=2) as xt_pool,
            tc.tile_pool(name="qk", bufs=2) as qk_pool,
            tc.tile_pool(name="vtp", bufs=1) as vt_pool,
            tc.tile_pool(name="vp", bufs=2) as vp_pool,
            tc.tile_pool(name="pp", bufs=4) as p_pool,
            tc.tile_pool(name="zt", bufs=1) as zt_pool,
            tc.tile_pool(name="sr", bufs=1) as sr_pool,
            tc.tile_pool(name="rc", bufs=2) as rc_pool,
            tc.tile_pool(name="ob", bufs=2) as out_pool,
            tc.tile_pool(name="ps_s", bufs=4, space="PSUM") as ps_s,
            tc.tile_pool(name="ps_a", bufs=2, space="PSUM") as ps_a,
            tc.tile_pool(name="ps_z", bufs=2, space="PSUM") as ps_z,
        ):
            id_sb = wpool.tile([128, 128], F32, name="id_sb")
            nc.scalar.dma_start(id_sb[:], ident[:])
            mask_sb = wpool.tile([128, 128], FP16, name="mask_sb")
            nc.scalar.dma_start(mask_sb[:], maskd[:])

            st = [dict() for _ in range(HPC)]   # per-head live tiles

            TENS = {"q": (xq, wq, bq), "k": (xk, wk, bk), "v": (xv, wv, bv)}

            def emit_loads(h):
                """Big X loads on the (pure) sync ring; weights on scalar ring."""
                for t in ("q", "k", "v"):
                    xd, wd, bd = TENS[t]
                    xh = xt_pool.tile([128, MT, S], F32R,
                                      name=f"x{t}{h}", tag="xt")
                    for n in range(4):
                        nc.sync.dma_start(
                            xh[:, :, bass.ts(n, 512)],
                            xd[h].rearrange("(a p) s -> p a s", p=128)
                                 [:, :, bass.ts(n, 512)])
                    wt = wpool.tile([128, MT, DH], F32R, name=f"w{t}{h}")
                    nc.scalar.dma_start(wt[:], wd[h].rearrange("a p d -> p a d"))
                    bt = wpool.tile([DH, 1], F32, name=f"b{t}{h}")
                    nc.scalar.dma_start(bt[:], bd[h])
                    st[h][f"x{t}"] = xh
                    st[h][f"w{t}"] = wt
                    st[h][f"b{t}"] = bt

            def emit_proj(h, t):
                """One tensor's projection chains; frees its X slot at the end."""
                if t == "q":
                    dst = qk_pool.tile([DH, S], F32R, name=f"qT{h}", tag="qT")
                    st[h]["qT"] = dst
                elif t == "k":
                    dst = qk_pool.tile([DH, S], F32R, name=f"kT{h}", tag="kT")
                    st[h]["kT"] = dst
                else:
                    dst = vt_pool.tile([DH, S], F32, name=f"vT{h}", tag="vT")
                    st[h]["vT"] = dst
                xh, wt, bt = st[h][f"x{t}"], st[h][f"w{t}"], st[h][f"b{t}"]
                for c in range(4):
                    acc = ps_s.tile([DH, 512], F32, name=f"acc{t}{h}{c}", tag="s")
                    for mt in range(MT):
                        nc.tensor.matmul(
                            acc[:], wt[:, mt, :],
                            xh[:, mt, bass.ts(c, 512)],
                            start=(mt == 0), stop=(mt == MT - 1))
                    nc.vector.tensor_scalar_add(
                        dst[:, bass.ts(c, 512)], acc[:], bt[:])

            def emit_vp(h):
                vT = st[h]["vT"]
                vp = vp_pool.tile([128, DH + 1, NT], FP16, name=f"vp{h}", tag="vp")
                nc.scalar.dma_start(vp[:, DH, :], ones16[:])
                for i in range(NT):
                    v_ps = ps_s.tile([128, DH], F32, name=f"vps{h}{i}", tag="s")
                    nc.tensor.transpose(v_ps[:], vT[:, bass.ts(i, 128)],
                                        id_sb[0:DH, 0:DH])
                    nc.vector.tensor_copy(vp[:, 0:DH, i], v_ps[:])
                st[h]["vp"] = vp

            def emit_B(h, interleave=None):
                """Causal attention: chunk-major with S^T lookahead.
                interleave[c] (optional) emits next-head work after chunk c."""
                qT, kT, vp = st[h]["qT"], st[h]["kT"], st[h]["vp"]
                zT = zt_pool.tile([DH + 1, S], F32R, name=f"zT{h}", tag="zT")
                srow = sr_pool.tile([DH + 1, S], F32, name=f"srow{h}", tag="srow")
                rc = rc_pool.tile([128, NT], F32, name=f"rc{h}", tag="rc")
                wot = wpool.tile([DH + 1, DM], F32R, name=f"wo{h}")
                nc.scalar.dma_start(wot[:], wo[h])
                st[h]["wo"] = wot

                def emit_rc(c):
                    for j in range(4 * c, 4 * c + 4):
                        rc_ps = ps_s.tile([128, 1], F32, name=f"rcp{h}{j}", tag="s")
                        nc.tensor.transpose(
                            rc_ps[:], srow[DH:DH + 1, bass.ts(j, 128)],
                            id_sb[DH:DH + 1, DH:DH + 1])
                        nc.vector.reciprocal(rc[:, j:j + 1], rc_ps[:])

                for c in range(4):
                    z_ps = ps_z.tile([DH + 1, 512], F32, name=f"z{h}{c}", tag="z")
                    n_i = 4 * c + 4

                    def emit_S(i):
                        qlo = max(512 * c, 128 * i)
                        w = 512 * (c + 1) - qlo
                        s_ps = ps_s.tile([128, 512], F32,
                                         name=f"s{h}{c}{i}", tag="s")
                        nc.tensor.matmul(s_ps[:, 0:w], kT[:, bass.ts(i, 128)],
                                         qT[:, qlo:qlo + w], start=True, stop=True)
                        P = p_pool.tile([128, 512], FP16,
                                        name=f"P{h}{c}{i}", tag="P")
                        nc.scalar.activation(P[:, 0:w], s_ps[:, 0:w],
                                             AF.Exp, scale=0.125)
                        if qlo == 128 * i:
                            nc.vector.tensor_mul(P[:, 0:128], P[:, 0:128],
                                                 mask_sb[:])
                        return (qlo, w, P)

                    staged = [emit_S(i) for i in range(min(LOOKAHEAD, n_i))]
                    if c > 0:
                        emit_rc(c - 1)   # previous chunk's recip columns
                    for i in range(n_i):
                        if i + LOOKAHEAD < n_i:
                            staged.append(emit_S(i + LOOKAHEAD))
                        qlo, w, P = staged[i]
                        nc.tensor.matmul(
                            z_ps[:, qlo - 512 * c: qlo - 512 * c + w],
                            vp[:, :, i], P[:, 0:w],
                            start=(i == 0), stop=(i == n_i - 1))
                    nc.vector.tensor_copy(zT[:, bass.ts(c, 512)], z_ps[:])
                    nc.vector.tensor_copy(srow[DH:DH + 1, bass.ts(c, 512)],
                                          z_ps[DH:DH + 1, :])
                    if interleave and c in interleave:
                        interleave[c]()
                emit_rc(3)
                st[h].update(zT=zT, rc=rc)

            def emit_C(h, quarters=(0, 1, 2, 3)):
                """Output projection + per-row softmax scale + store."""
                zT, rc, wot = st[h]["zT"], st[h]["rc"], st[h]["wo"]
                for quarter in quarters:
                    ob = out_pool.tile([128, 4, DM], F32, name=f"ob{h}{quarter}",
                                       tag="ob")
                    for a in range(4):
                        j = 4 * quarter + a
                        for (mo, mw) in ((0, 512), (512, 256)):
                            a_ps = ps_a.tile([128, 512], F32,
                                             name=f"a{h}{j}{mo}", tag="a")
                            nc.tensor.matmul(a_ps[:, 0:mw],
                                             zT[:, bass.ts(j, 128)],
                                             wot[:, mo:mo + mw],
                                             start=True, stop=True)
                            if mo == 0:
                                nc.scalar.activation(ob[:, a, mo:mo + mw],
                                                     a_ps[:, 0:mw], AF.Copy,
                                                     scale=rc[:, j:j + 1])
                            else:
                                nc.vector.tensor_scalar_mul(ob[:, a, mo:mo + mw],
                                                            a_ps[:, 0:mw],
                                                            rc[:, j:j + 1])
                    nc.gpsimd.dma_start(
                        out[h, bass.ts(quarter, 512), :]
                           .rearrange("(a p) m -> p a m", p=128),
                        ob[:])

            emit_loads(0)
            for t in ("q", "k", "v"):
                emit_proj(0, t)
            emit_vp(0)
            for h in range(HPC):
                nxt = h + 1
                acts = {0: [], 1: [], 2: [], 3: []}
                if nxt < HPC:
                    emit_loads(nxt)
                    acts[0].append(lambda n=nxt: emit_proj(n, "q"))
                    acts[1].append(lambda n=nxt: emit_proj(n, "k"))
                    acts[2].append(lambda n=nxt: emit_proj(n, "v"))
                    acts[2].append(lambda n=nxt: emit_vp(n))
                if h >= 1:
                    acts[0].append(lambda p=h - 1: emit_C(p, (0, 1)))
                    acts[1].append(lambda p=h - 1: emit_C(p, (2,)))
                    acts[2].append(lambda p=h - 1: emit_C(p, (3,)))
                inter = {c: (lambda fs=fs: [f() for f in fs])
                         for c, fs in acts.items() if fs}
                emit_B(h, interleave=inter)
            emit_C(HPC - 1)
    nc.compile()
    return nc


_CACHED = None


def _program():
    global _CACHED
    if _CACHED is None:
        _CACHED = build_program()
    return _CACHED


def _make_in_maps(inputs):
    xq_f = np.asarray(inputs["normalized_resid_pre_q"], dtype=np.float32)
    xk_f = np.asarray(inputs["normalized_resid_pre_k"], dtype=np.float32)
    xv_f = np.asarray(inputs["normalized_resid_pre_v"], dtype=np.float32)
    WQ = np.asarray(inputs["W_Q"], dtype=np.float32)
    WK = np.asarray(inputs["W_K"], dtype=np.float32)
    WV = np.asarray(inputs["W_V"], dtype=np.float32)
    WO = np.asarray(inputs["W_O"], dtype=np.float32)
    bQ = np.asarray(inputs["b_Q"], dtype=np.float32)
    bK = np.asarray(inputs["b_K"], dtype=np.float32)
    bV = np.asarray(inputs["b_V"], dtype=np.float32)
    bO = np.asarray(inputs["b_O"], dtype=np.float32)

    ident = np.eye(128, dtype=np.float32)
    maskd = (np.arange(128)[:, None] <= np.arange(128)[None, :]).astype(np.float16)
    ones16 = np.ones((128, NT), np.float16)

    in_maps = []
    for c in range(N_CORES):
        b = c % 2
        hg = c // 2
        hs = slice(HPC * hg, HPC * hg + HPC)
        m = {
            "xq": np.ascontiguousarray(xq_f[b, :, hs, :].transpose(1, 2, 0)),
            "xk": np.ascontiguousarray(xk_f[b, :, hs, :].transpose(1, 2, 0)),
            "xv": np.ascontiguousarray(xv_f[b, :, hs, :].transpose(1, 2, 0)),
            "wq": np.ascontiguousarray(WQ[hs].reshape(HPC, MT, 128, DH)),
            "wk": np.ascontiguousarray(WK[hs].reshape(HPC, MT, 128, DH)),
            "wv": np.ascontiguousarray(WV[hs].reshape(HPC, MT, 128, DH)),
            "wo": np.ascontiguousarray(np.concatenate(
                [WO[hs], np.broadcast_to(bO / H, (HPC, 1, DM))], axis=1)),
            "bq": np.ascontiguousarray(bQ[hs].reshape(HPC, DH, 1)),
            "bk": np.ascontiguousarray(bK[hs].reshape(HPC, DH, 1)),
            "bv": np.ascontiguousarray(bV[hs].reshape(HPC, DH, 1)),
            "ident": ident,
            "maskd": maskd,
            "ones16": ones16,
        }
        in_maps.append(m)
    return in_maps


def run(inputs, trace=False, **kw):
    nc = _program()
    in_maps = _make_in_maps(inputs)
    res = run_bass_kernel_spmd(nc, in_maps, core_ids=list(range(N_CORES)),
                               trace=trace, **kw)
    full = np.zeros((B, S, H, DM), np.float32)
    for c in range(N_CORES):
        b = c % 2
        hg = c // 2
        o = res.results[c]["out"]
        for j in range(HPC):
            full[b, :, HPC * hg + j, :] = o[j]
    return full, res


def kernel(**inputs):
    full, _ = run(inputs)
    return full
